# revision 132
# baseline (speedup 1.0000x reference)
"""EquivariantAttention Trainium2 kernel.

B=8 batches data-parallel over 8 NeuronCores; per core:
  qkv = x @ W_qkv + b_qkv ; dist = cdist(g, g)
  S^T[j,i] = (q_i.k_j) * exp(-dist)/sqrt(H)   (transposed: j on partitions)
  U^T = exp(S^T); out^T[h,i] = V^T @ U^T / l_i ; y = out @ W_out + b_out

q,k are stored fp8e4 so the N^2 score matmul runs in DoubleRow perf mode
(K=256 per matmul, half the cycles); 1/sqrt(H) is folded into
E = exp(-dist - ln(sqrt(H))) as an exp bias so q,k keep natural scale.
attn@V and projections stay bf16/f32r: fp8 there fails the 2e-2 budget
(the output is a near-cancelling weighted mean; element rel-errors do
not average down). x^T and W_qkv are bf16 (error contribution ~1e-3).

The main loop is software-pipelined across i-chunks: iteration `it`
interleaves, per j-tile step, the DR score matmuls + score*E (DVE) +
expU (ACT) for chunk `it`, the d2 matmuls + clamps for chunk `it+1`,
and the attn@V + out-projection for chunk `it-1`, so PE always has
independent work while DVE drains the score PSUMs. Iteration 0 uses the
v-projection as its PE filler. The d2 clamp on DVE is mandatory on HW
(f32r matmul noise drives d2 negative near the diagonal; sqrt would
NaN) and doubles as the PSUM->SBUF move into the E tile (bf16), where
sqrt and exp then run in place. The ACT chain per iteration is
[expU x8][sqrt x8][expE x4]: table sets swap twice per iteration.
"""

import numpy as np

import concourse.bass as bass
from concourse import bacc
import concourse.mybir as mybir
import concourse.tile as tile
from concourse.masks import make_identity
from concourse.tile import add_dep_helper

P = 128
H = 512
SC = 512
HT = H // P  # 4

f32 = mybir.dt.float32
f32r = mybir.dt.float32r
bf16 = mybir.dt.bfloat16
f8 = mybir.dt.float8e4
AF = mybir.ActivationFunctionType
OP = mybir.AluOpType
DR = mybir.MatmulPerfMode.DoubleRow
LN_SQRT_H = 0.5 * float(np.log(H))


def _body(tc, n, x, g, wqkv, bqkv, wout, bout, y):
    nc = tc.nc
    NT = n // P
    NC_ = n // SC
    ITC = SC // P  # i-tiles per chunk (4)
    SPH = NT // HT  # AV j-steps per ht group

    with (
        nc.allow_low_precision(
            reason="fp8 q/k feed DoubleRow score matmuls; bf16 attn weights"
        ),
        tc.tile_pool(name="const", bufs=1) as const,
        tc.tile_pool(name="geo", bufs=1) as geo,
        tc.tile_pool(name="et_pool", bufs=2) as et_pool,
        tc.tile_pool(name="small", bufs=2) as small,
        tc.tile_pool(name="ps_s", bufs=2, space="PSUM") as ps_s,
        tc.tile_pool(name="ps_d", bufs=2, space="PSUM") as ps_d,
        tc.tile_pool(name="ps_o", bufs=3, space="PSUM") as ps_o,
        tc.tile_pool(name="ps_l", bufs=1, space="PSUM") as ps_l,
    ):
        # ---- constants; DMA order matters: g and the first x group gate
        # the PE pipeline, weights ride the gpsimd queue ----
        bqk_sb = const.tile([P, 8], f32)  # cols 0-3: b_q m-tiles, 4-7: b_k
        bo_bc = const.tile([P, H], f32)
        ones_bf = const.tile([P, 1], bf16)
        nc.vector.memset(ones_bf, 1.0)
        ones_row = const.tile([1, P], f32r)
        nc.vector.memset(ones_row.bitcast(f32), 1.0)
        ebias = const.tile([P, 1], f32)  # exp bias: fold 1/sqrt(H) into E
        nc.vector.memset(ebias, -LN_SQRT_H)

        # augmented geometry, transposed: d2[j,i] = sum_k h_k[j] * g_k[i].
        # f32r, NOT bf16: absolute bf16 error on |g|^2 (~15) is amplified
        # by sqrt near d2=0 and costs ~0.5% output error.
        hT8 = geo.tile([8, n], f32r)
        gT8 = geo.tile([8, n], f32r)

        with (
            tc.tile_pool(name="qkv", bufs=1) as qkv,
            tc.tile_pool(name="e_pool", bufs=2) as e_pool,
            tc.tile_pool(name="ut_pool", bufs=2) as ut_pool,
            tc.tile_pool(name="ot_pool", bufs=1) as ot_pool,
            tc.tile_pool(name="xt_pool", bufs=1) as xt_pool,
        ):
            qT = qkv.tile([P, HT, n], f8)  # q^T (natural scale), [h, i]
            kT = qkv.tile([P, HT, n], f8)  # k^T, [h, j]
            v_bf = qkv.tile([P, NT, H], bf16)  # v natural, [j, h]
            wout_sb = qkv.tile([P, HT, H], f32r)
            wqkv_bf = qkv.tile([P, HT, 3 * H], bf16)
            bv_row = qkv.tile([1, H], f32r)  # v bias as K=1 matmul operand
            xT = xt_pool.tile([P, HT, n], bf16)
            # f32 identity: a bf16 identity with f32(r) data crashes the
            # exec unit on HW (mixed-dtype transpose), don't try it
            ident = xt_pool.tile([P, P], f32)
            make_identity(nc, ident)

            # ---- ACT chain helper: keeps sqrt/exp table-set switches at
            # two per iteration by pinning ACT emission order ----
            state = {"prev": None}

            def chain(a):
                if state["prev"] is not None:
                    add_dep_helper(
                        a.ins,
                        state["prev"].ins,
                        sync=False,
                        reason="ACT table-set batching",
                    )
                state["prev"] = a
                return a

            Es = {}

            def emit_d2(ic, jt):
                # one d2 matmul + DVE clamp into the E tile (bf16). The
                # clamp is mandatory on HW (f32r noise -> negative d2 ->
                # sqrt NaN) and doubles as the PSUM->SBUF move.
                isl = slice(ic * SC, (ic + 1) * SC)
                if ic not in Es:
                    Etile = e_pool.tile([P, NT, SC], bf16, tag="E")
                    Es[ic] = Etile
                dist = Es[ic]
                psd = ps_d.tile([P, SC], f32, tag="psd")
                nc.tensor.matmul(
                    psd,
                    lhsT=hT8[:, jt * P : (jt + 1) * P],
                    rhs=gT8[:, isl],
                    start=True,
                    stop=True,
                )
                nc.vector.tensor_scalar_max(dist[:, jt, :], psd, 0.0)

            def emit_sqrt_exp(ic):
                # in-place on the E tile: sqrt pairs, then exp quads with
                # the -ln(sqrt(H)) bias folded in
                E = Es[ic]
                for jp in range(0, NT, 4):
                    chain(
                        nc.scalar.activation(
                            E[:, jp : jp + 4, :], E[:, jp : jp + 4, :], AF.Sqrt
                        )
                    )
                for jp in range(0, NT, 4):
                    chain(
                        nc.scalar.activation(
                            E[:, jp : jp + 4, :],
                            E[:, jp : jp + 4, :],
                            AF.Exp,
                            scale=-1.0,
                            bias=ebias,
                        )
                    )

            # ---- prologue: geometry prep, x transposes, W staging ----
            with tc.tile_pool(name="wstage", bufs=1) as wstage:
                g_sb = wstage.tile([P, NT, 3], f32)
                # The DMA engine pool is effectively serial: x0 must be the
                # very first sizable transfer so PE starts transposing at
                # ~2.5us; W_qkv rides in quarters interleaved between x
                # groups (each quarter converts to bf16 on Pool while the
                # next x group transfers); constants trail the x stream.
                # q/k projection blocks are emitted INSIDE the x loop as
                # soon as their x-chunk and W-quarter have been requested,
                # so PE computes behind the serial DMA stream instead of
                # idling until everything lands.
                wq_r = wqkv.rearrange("(kt p) m -> p kt m", p=P)
                WQ = 3 * H // 4
                NH = max(1, NT // 8)
                NG = NT // NH
                d2q = list(range(NT))

                def emit_w_quarter(wh):
                    msl = slice(wh * WQ, (wh + 1) * WQ)
                    w_q = wstage.tile([P, HT, WQ], f32, tag=f"wq{wh % 2}")
                    nc.gpsimd.dma_start(w_q, wq_r[:, :, msl])
                    nc.gpsimd.tensor_copy(wqkv_bf[:, :, msl], w_q)

                def emit_qk_block(c, mt):
                    dst = qT if mt < 4 else kT
                    mi = mt % 4
                    ps = ps_s.tile([P, SC], f32, tag="pss")
                    for kc in range(HT):
                        nc.tensor.matmul(
                            ps,
                            lhsT=wqkv_bf[:, kc, mt * P : (mt + 1) * P],
                            rhs=xT[:, kc, c * SC : (c + 1) * SC],
                            start=(kc == 0),
                            stop=(kc == HT - 1),
                        )
                    if d2q and (mt + c) % 2 == 1:
                        emit_d2(0, d2q.pop(0))
                    dap = dst[:, mi, c * SC : (c + 1) * SC]
                    if (mt + c) % 2 == 0:
                        nc.scalar.activation(
                            dap, ps, AF.Identity, bias=bqk_sb[:, mt : mt + 1]
                        )
                    else:
                        nc.vector.tensor_scalar_add(
                            dap, ps, bqk_sb[:, mt : mt + 1]
                        )

                # static schedule: after group qi, which W quarters to
                # request and which (c, mt) blocks are fully fed.
                # x-chunk c completes with group (4*(c+1))//NH - 1;
                # W quarter wh is requested after group 2*wh+1 (or at the
                # end) and q/k block (c, mt) needs quarter mt//3.
                wq_after = {}
                for wh in range(4):
                    gidx = 2 * wh + 1 if 2 * wh + 1 < NG else NG - 1
                    wq_after.setdefault(gidx, []).append(wh)
                blk_after = {}
                for c in range(NC_):
                    gx = (4 * (c + 1)) // NH - 1
                    for mt in range(8):
                        wh = min(mt // 3, 3)
                        gw = 2 * wh + 1 if 2 * wh + 1 < NG else NG - 1
                        blk_after.setdefault(max(gx, gw), []).append((c, mt))

                x_r = x.rearrange("(nt p) h -> p nt h", p=P)
                with tc.tile_pool(name="xsb_pool", bufs=3) as xsb_pool:
                    for qi, hh in enumerate(range(0, NT, NH)):
                        x_sb = xsb_pool.tile([P, NH, H], f32, tag="x_sb")
                        eng = nc.gpsimd if qi == 0 else (
                            nc.scalar, nc.sync
                        )[qi % 2]
                        eng.dma_start(x_sb, x_r[:, hh : hh + NH, :])
                        if qi == 0:
                            # g is tiny so it can trail x0 on the serial
                            # DMA engine without real cost
                            nc.sync.dma_start(
                                g_sb, g.rearrange("(nt p) c -> p nt c", p=P)
                            )
                        if qi == min(1, NG - 1):
                            nc.sync.dma_start(
                                bqk_sb,
                                bqkv[0 : 2 * H].rearrange("(mt p) -> p mt", p=P),
                            )
                        for wh in wq_after.get(qi, []):
                            emit_w_quarter(wh)
                        for nt in range(NH):
                            for ht in range(HT):
                                tp_pool, tp_tag = (
                                    (ps_o, "pso"),
                                    (ps_s, "pss"),
                                    (ps_d, "psd"),
                                )[(nt * HT + ht) % 3]
                                pt = tp_pool.tile([P, SC], f32, tag=tp_tag)
                                nc.tensor.transpose(
                                    pt[:, :P],
                                    x_sb[:, nt, ht * P : (ht + 1) * P],
                                    ident,
                                )
                                dst_ap = xT[
                                    :, ht, (hh + nt) * P : (hh + nt + 1) * P
                                ]
                                if ht % 2 == 0:
                                    nc.scalar.copy(dst_ap, pt[:, :P])
                                else:
                                    nc.vector.tensor_copy(dst_ap, pt[:, :P])
                        if qi == 0:
                            # geometry prep as PE filler while x1 transfers
                            g2 = wstage.tile([P, NT, 3], f32)
                            nc.vector.tensor_mul(g2, g_sb, g_sb)
                            sq = wstage.tile([P, NT, 1], f32)
                            nc.vector.reduce_sum(
                                sq, g2, axis=mybir.AxisListType.X
                            )
                            Ag = wstage.tile([P, NT, 8], f32)
                            Ah = wstage.tile([P, NT, 8], f32)
                            nc.vector.memset(Ag, 0.0)
                            nc.vector.memset(Ah, 0.0)
                            nc.vector.tensor_copy(Ag[:, :, 0:3], g_sb)
                            nc.vector.tensor_copy(Ag[:, :, 3:4], sq)
                            nc.vector.memset(Ag[:, :, 4:5], 1.0)
                            nc.vector.tensor_scalar_mul(
                                Ah[:, :, 0:3], g_sb, -2.0
                            )
                            nc.vector.memset(Ah[:, :, 3:4], 1.0)
                            nc.vector.tensor_copy(Ah[:, :, 4:5], sq)
                            for nt in range(NT):
                                pt = ps_s.tile([P, SC], f32, tag="pss")
                                nc.tensor.transpose(
                                    pt[:8, :P], Ah[:, nt, :], ident
                                )
                                nc.scalar.copy(
                                    hT8[:, nt * P : (nt + 1) * P], pt[:8, :P]
                                )
                                pt2 = ps_d.tile([P, SC], f32, tag="psd")
                                nc.tensor.transpose(
                                    pt2[:8, :P], Ag[:, nt, :], ident
                                )
                                nc.scalar.copy(
                                    gT8[:, nt * P : (nt + 1) * P], pt2[:8, :P]
                                )
                        for c, mt in blk_after.get(qi, []):
                            emit_qk_block(c, mt)

                nc.gpsimd.dma_start(bo_bc, bout.partition_broadcast(P))
                nc.gpsimd.dma_start(
                    bv_row,
                    bqkv[2 * H : 3 * H]
                    .rearrange("(o m) -> o m", o=1)
                    .bitcast(f32r),
                )

            while d2q:
                emit_d2(0, d2q.pop(0))
            emit_sqrt_exp(0)
            # wout DMA after the q/k phase: first use is the chunk-0
            # out-projection one pipeline iteration later
            nc.gpsimd.dma_start(
                wout_sb,
                wout.rearrange("(kt p) m -> p kt m", p=P).bitcast(f32r),
            )


            # ---- pipelined main loop over i-chunks ----
            y_r = y.rearrange("(nt p) h -> p nt h", p=P)
            UTs = {}
            lbcs = {}

            for it in range(NC_ + 1):
                cur, prv, nxt = it, it - 1, it + 1
                E = Es.pop(cur) if cur < NC_ else None
                if cur < NC_:
                    UTc = ut_pool.tile([P, NT, SC], bf16, tag="UT")
                    UTs[cur] = UTc
                    psl = ps_l.tile([1, SC], f32, tag="psl")
                    isl = slice(cur * SC, (cur + 1) * SC)
                if prv >= 0:
                    UTp = UTs.pop(prv)
                    lbc = lbcs.pop(prv)
                    outT = ot_pool.tile([P, HT, SC], f32r, tag="outT")
                for jt in range(NT):
                    if cur < NC_:
                        jsl = slice(jt * P, (jt + 1) * P)
                        pss = ps_s.tile([P, SC], f32, tag="pss")
                        for kc in (0, 2):
                            nc.tensor.matmul(
                                pss,
                                lhsT=kT[:, kc : kc + 2, jsl],
                                rhs=qT[:, kc : kc + 2, isl],
                                start=(kc == 0),
                                stop=(kc == 2),
                                perf_mode=DR,
                            )
                    if cur < NC_:
                        # score*E + expU right after QK: this is the
                        # iteration's critical path, so it goes first in
                        # the in-order DVE queue (clamps/outT are not
                        # urgent and follow)
                        if jt % 2 == 0:
                            et2 = et_pool.tile([P, 2, SC], f32, tag="et")
                        nc.vector.tensor_mul(et2[:, jt % 2, :], pss, E[:, jt, :])
                        if jt % 2 == 1:
                            chain(
                                nc.scalar.activation(
                                    UTc[:, jt - 1 : jt + 1, :], et2, AF.Exp
                                )
                            )
                    if it == 0:
                        # v natural [j, h] (bf16) as iteration-0 PE filler;
                        # bias via a K=1 ones-matmul so the PSUM->SBUF copy
                        # can alternate between ACT and the loaded DVE
                        psv = ps_o.tile([P, SC], f32, tag="pso")
                        for kc in range(HT):
                            nc.tensor.matmul(
                                psv,
                                lhsT=xT[:, kc, jt * P : (jt + 1) * P],
                                rhs=wqkv_bf[:, kc, 2 * H : 3 * H],
                                start=(kc == 0),
                                stop=False,
                            )
                        nc.tensor.matmul(
                            psv, lhsT=ones_row, rhs=bv_row,
                            start=False, stop=True,
                        )
                        if jt % 2 == 0:
                            nc.scalar.copy(v_bf[:, jt, :], psv)
                        else:
                            nc.vector.tensor_copy(v_bf[:, jt, :], psv)
                    if prv >= 0:
                        # attn@V for prv: ht-major, SPH j-steps per ht
                        ht = jt // SPH
                        j0 = (jt % SPH) * HT
                        if j0 == 0:
                            pso = ps_o.tile([P, SC], f32, tag="pso")
                        for jtt in range(j0, j0 + HT):
                            nc.tensor.matmul(
                                pso,
                                lhsT=v_bf[:, jtt, ht * P : (ht + 1) * P],
                                rhs=UTp[:, jtt, :],
                                start=(jtt == 0),
                                stop=(jtt == NT - 1),
                            )
                        if j0 + HT == NT:
                            nc.vector.tensor_mul(outT[:, ht, :], pso, lbc)
                    if nxt < NC_:
                        emit_d2(nxt, jt)
                if cur < NC_:
                    # row-sum matmuls batched after the step loop: inline
                    # they make the in-order PE queue wait on ACT's expU
                    for jt in range(NT):
                        nc.tensor.matmul(
                            psl,
                            lhsT=ones_bf,
                            rhs=UTc[:, jt, :],
                            start=(jt == 0),
                            stop=(jt == NT - 1),
                        )
                    # 1/l broadcast for chunk cur (used next iteration)
                    linv_row = et_pool.tile([1, SC], f32r, tag="linv")
                    nc.vector.reciprocal(linv_row, psl)
                    psb = ps_d.tile([P, SC], f32, tag="psd")
                    nc.tensor.matmul(
                        psb, lhsT=ones_row, rhs=linv_row, start=True, stop=True
                    )
                    lbc_c = et_pool.tile([P, SC], f32, tag="lbc")
                    nc.vector.tensor_copy(lbc_c, psb)
                    lbcs[cur] = lbc_c
                if nxt < NC_:
                    emit_sqrt_exp(nxt)
                if prv >= 0:
                    # out-projection + bias + store for chunk prv
                    for it4 in range(ITC):
                        psy = ps_o.tile([P, SC], f32, tag="pso")
                        for ht in range(HT):
                            nc.tensor.matmul(
                                psy,
                                lhsT=outT[:, ht, it4 * P : (it4 + 1) * P],
                                rhs=wout_sb[:, ht, :],
                                start=(ht == 0),
                                stop=(ht == HT - 1),
                            )
                        if it == NC_ and it4 >= 2:
                            # final drain: borrow the retired lbc ring so
                            # all four stores use distinct buffers and no
                            # ysb waits on a prior store's DMA readback
                            ysb = et_pool.tile([P, H], f32, tag="lbc")
                        else:
                            ysb = small.tile([P, H], f32, tag="ysb")
                        nc.vector.tensor_add(ysb, psy, bo_bc)
                        nc.sync.dma_start(y_r[:, prv * ITC + it4, :], ysb)



def build_bass(n: int = 2048) -> bass.Bass:
    nc = bacc.Bacc(None, target_bir_lowering=False)
    x = nc.dram_tensor("x", [n, H], f32, kind="ExternalInput")[:, :]
    g = nc.dram_tensor("g", [n, 3], f32, kind="ExternalInput")[:, :]
    wqkv = nc.dram_tensor("w_qkv", [H, 3 * H], f32, kind="ExternalInput")[:, :]
    bqkv = nc.dram_tensor("b_qkv", [3 * H], f32, kind="ExternalInput")[:]
    wout = nc.dram_tensor("w_out", [H, H], f32, kind="ExternalInput")[:, :]
    bout = nc.dram_tensor("b_out", [H], f32, kind="ExternalInput")[:]
    y = nc.dram_tensor("y", [n, H], f32, kind="ExternalOutput")[:, :]
    with tile.TileContext(nc) as tc:
        _body(tc, n, x, g, wqkv, bqkv, wout, bout, y)
    nc.finalize()
    return nc


_CACHED = {}


def _get_nc(n: int = 2048) -> bass.Bass:
    if n not in _CACHED:
        _CACHED[n] = build_bass(n)
    return _CACHED[n]


def kernel(**inputs) -> np.ndarray:
    from concourse.bass_utils import run_bass_kernel_spmd

    x = np.ascontiguousarray(inputs["x"], dtype=np.float32)
    g = np.ascontiguousarray(inputs["geometric_features"], dtype=np.float32)
    wqkv = np.ascontiguousarray(inputs["W_qkv"], dtype=np.float32)
    bqkv = np.ascontiguousarray(inputs["b_qkv"], dtype=np.float32)
    wout = np.ascontiguousarray(inputs["W_out"], dtype=np.float32)
    bout = np.ascontiguousarray(inputs["b_out"], dtype=np.float32)

    B, n, _ = x.shape
    nc = _get_nc(n)
    core_ids = list(range(B))
    in_maps = [
        {
            "x": np.ascontiguousarray(x[b]),
            "g": np.ascontiguousarray(g[b]),
            "w_qkv": wqkv,
            "b_qkv": bqkv,
            "w_out": wout,
            "b_out": bout,
        }
        for b in range(B)
    ]
    res = run_bass_kernel_spmd(nc, in_maps, core_ids)
    return np.stack([res.results[b]["y"] for b in range(B)]).astype(np.float32)
